# revision 6
# baseline (speedup 1.0000x reference)
"""GAT (2-layer, 8 heads) + MLP on 8 Trainium2 NeuronCores — full on-device
Bass/Tile pipeline.

Strategy (graph/data parallel, dst-sharded, per the sharding hint):
  Edges (incl. self-loops) are sorted by dst and packed into uniform tiles of
  4 groups x 128 slots; no dst-segment crosses a 128-slot group boundary.
  Per edge tile, on device:
    - indirect-DMA gather of source-node rows (x for layer 1, h1r for layer 2)
    - TensorE transposes -> k-major blocks -> node-transform matmuls (h|alpha_src)
    - alpha_dst via per-segment indirect gather + segment-expand matmul (E)
    - segment softmax without max-subtraction (exp of small logits), segment
      sums via selection-matrix (S) matmuls, normalize, weighted aggregation
    - masked indirect-DMA scatter of one representative row per segment
  Between layers: 8-core AllGather of the h1r node table (halo exchange).
  The MLP runs node-major on each core's shard; biases/means are folded into
  the weights host-side where exact.
"""
import sys
import time

for p in ("/opt/trn_rl_repo",):
    if p not in sys.path:
        sys.path.append(p)

import numpy as np
from dataclasses import dataclass

N_CORES = 8
N_REAL = 50000
NSHARD = 6272  # 8 * 6272 = 50176 >= 50001


@dataclass
class Cfg:
    n_real: int
    n_cores: int
    nshard: int
    T: int
    IN: int = 128
    A: int = 8
    CH1: int = 32
    CH2: int = 64
    G: int = 4

    @property
    def npad(self):
        return self.n_cores * self.nshard

    @property
    def C1(self):
        return self.A * self.CH1  # 256

    @property
    def C2(self):
        return self.A * self.CH2  # 512


# ---------------------------------------------------------------- host tiling


def build_tiles(src_sorted, dst_sorted, lo, hi, cfg):
    """Pack this core's dst-sorted edges into tiles.

    meta_i cols: 0:4 src index per group, 4:8 scatter (wmask) index per group,
                 8 adst (alpha-gather row, indexed by sid on partitions).
    """
    G, P = cfg.G, 128
    sent = cfg.n_real
    BIG = cfg.nshard  # masked scatter index: one past the last valid row
    counts = np.bincount(dst_sorted - lo, minlength=hi - lo)
    starts = np.zeros(hi - lo + 1, dtype=np.int64)
    np.cumsum(counts, out=starts[1:])

    tiles_i, tiles_c, tiles_r = [], [], []

    def new_tile():
        mi = np.empty((P, 9), dtype=np.int32)
        mi[:, 0:4] = sent
        mi[:, 4:8] = BIG
        mi[:, 8] = 0
        mc = np.full((P, G), 127.0, dtype=np.float32)
        mr = np.full((G, P), 127.0, dtype=np.float32)
        return [mi, mc, mr, 0]  # [3] = nseg

    cur = new_tile()
    g_idx, fill = 0, 0

    def close_tile():
        nonlocal cur, g_idx, fill
        tiles_i.append(cur[0])
        tiles_c.append(cur[1])
        tiles_r.append(cur[2].reshape(-1))
        cur = new_tile()
        g_idx, fill = 0, 0

    for dl in range(hi - lo):
        k = int(counts[dl])
        if k == 0:
            continue
        assert k <= P, f"degree {k} exceeds group size"
        if fill + k > P:
            g_idx += 1
            fill = 0
            if g_idx == G:
                close_tile()
        if cur[3] == 126:
            close_tile()
        mi, mc, mr, nseg = cur
        sid = nseg
        cur[3] = nseg + 1
        mi[sid, 8] = dl
        s0 = int(starts[dl])
        sl = slice(fill, fill + k)
        mi[sl, g_idx] = src_sorted[s0:s0 + k]
        mi[fill, 4 + g_idx] = dl
        mc[sl, g_idx] = float(sid)
        mr[g_idx, sl] = float(sid)
        fill += k
    if cur[3] > 0:
        close_tile()

    return tiles_i, tiles_c, tiles_r


def host_prep(x, edge_index, cfg):
    n = cfg.n_real
    src = np.concatenate([np.asarray(edge_index[0]), np.arange(n)]).astype(np.int64)
    dst = np.concatenate([np.asarray(edge_index[1]), np.arange(n)]).astype(np.int64)
    order = np.argsort(dst, kind="stable")
    src_s = src[order].astype(np.int32)
    dst_s = dst[order].astype(np.int32)

    xpad = np.zeros((cfg.npad, cfg.IN), dtype=np.float32)
    xpad[:n] = np.asarray(x, dtype=np.float32)

    all_tiles = []
    bounds = np.searchsorted(dst_s, np.arange(0, cfg.npad + 1, cfg.nshard))
    for c in range(cfg.n_cores):
        lo, hi = c * cfg.nshard, (c + 1) * cfg.nshard
        e0, e1 = bounds[c], bounds[c + 1]
        all_tiles.append(build_tiles(src_s[e0:e1], dst_s[e0:e1], lo, hi, cfg))

    T = max(len(t[0]) for t in all_tiles)
    cfg.T = T
    metas = []
    for ti, tc, tr in all_tiles:
        pad = T - len(ti)
        if pad:
            mi = np.empty((128, 9), dtype=np.int32)
            mi[:, 0:4] = cfg.n_real
            mi[:, 4:8] = cfg.nshard
            mi[:, 8] = 0
            ti = ti + [mi] * pad
            tc = tc + [np.full((128, 4), 127.0, np.float32)] * pad
            tr = tr + [np.full((512,), 127.0, np.float32)] * pad
        metas.append((np.stack(ti), np.stack(tc), np.stack(tr)))
    return xpad, metas


def prep_weights(W1, a1_src, a1_dst, W2, a2_src, a2_dst, Wm1, bm1, b2, Wm2, cfg):
    def blockdiag(a, ch):
        B = np.zeros((cfg.A * ch, cfg.A), dtype=np.float32)
        for h in range(cfg.A):
            B[h * ch:(h + 1) * ch, h] = a[h]
        return B

    W1 = np.asarray(W1, np.float32)
    W2 = np.asarray(W2, np.float32)
    Wm1 = np.asarray(Wm1, np.float32)
    W1aug = np.concatenate(
        [W1, W1 @ blockdiag(np.asarray(a1_src, np.float32), cfg.CH1)], axis=1)
    W1ad = W1 @ blockdiag(np.asarray(a1_dst, np.float32), cfg.CH1)
    W2as = W2 @ blockdiag(np.asarray(a2_src, np.float32), cfg.CH2)
    W2ad = W2 @ blockdiag(np.asarray(a2_dst, np.float32), cfg.CH2)
    bm1p = np.asarray(bm1, np.float32) + np.asarray(b2, np.float32) @ Wm1
    return dict(
        W1aug=W1aug, W1ad=W1ad, W2h=W2, W2as=W2as, W2ad=W2ad,
        Wm1=Wm1 / cfg.A,  # folds the head-mean 1/8
        bm1=bm1p.reshape(-1, 1).astype(np.float32),
        Wm2=np.asarray(Wm2, np.float32),
    )


# ------------------------------------------------------------- device program


DEBUG_OUTPUTS = False


def build_program(cfg):
    from concourse import bass, bacc, mybir
    import concourse.tile as tile
    from concourse.masks import make_identity

    f32 = mybir.dt.float32
    i32 = mybir.dt.int32
    P = 128
    A, C1, C2 = cfg.A, cfg.C1, cfg.C2
    NSH, NPAD, T, G = cfg.nshard, cfg.npad, cfg.T, cfg.G
    NB = NSH // P

    nc = bacc.Bacc("TRN2", target_bir_lowering=False, debug=False,
                   num_devices=cfg.n_cores)

    xpad = nc.dram_tensor("xpad", [NPAD, cfg.IN], f32, kind="ExternalInput")
    xshard = nc.dram_tensor("xshard", [NSH, cfg.IN], f32, kind="ExternalInput")
    meta_i = nc.dram_tensor("meta_i", [T, P, 9], i32, kind="ExternalInput")
    meta_c = nc.dram_tensor("meta_c", [T, P, G], f32, kind="ExternalInput")
    meta_r = nc.dram_tensor("meta_r", [T, G * P], f32, kind="ExternalInput")
    W1aug_d = nc.dram_tensor("W1aug", [cfg.IN, C1 + A], f32, kind="ExternalInput")
    W1ad_d = nc.dram_tensor("W1ad", [cfg.IN, A], f32, kind="ExternalInput")
    W2h_d = nc.dram_tensor("W2h", [C1, C2], f32, kind="ExternalInput")
    W2as_d = nc.dram_tensor("W2as", [C1, A], f32, kind="ExternalInput")
    W2ad_d = nc.dram_tensor("W2ad", [C1, A], f32, kind="ExternalInput")
    Wm1_d = nc.dram_tensor("Wm1", [64, 64], f32, kind="ExternalInput")
    bm1_d = nc.dram_tensor("bm1", [64, 1], f32, kind="ExternalInput")
    Wm2_d = nc.dram_tensor("Wm2", [64, 2], f32, kind="ExternalInput")

    ad1t = nc.dram_tensor("ad1t", [NSH, A], f32, kind="ExternalOutput")
    ad2t = nc.dram_tensor("ad2t", [NSH, A], f32, kind="ExternalOutput")
    t2shard = nc.dram_tensor("t2shard", [NSH, C1], f32, kind="Internal")
    t2dbg = (nc.dram_tensor("t2dbg", [NSH, C1], f32, kind="ExternalOutput")
             if DEBUG_OUTPUTS else None)
    t2full = nc.dram_tensor("t2full", [NPAD, C1], f32, kind="Internal",
                            addr_space="Shared")
    h2mt = nc.dram_tensor("h2mt", [NSH, 64], f32, kind="ExternalOutput")
    outf = nc.dram_tensor("outf", [NSH, 2], f32, kind="ExternalOutput")

    EXP = mybir.ActivationFunctionType.Exp
    RELU = mybir.ActivationFunctionType.Relu
    EQ = mybir.AluOpType.is_equal
    MUL = mybir.AluOpType.mult

    with tile.TileContext(nc) as tc:
        with (
            tc.tile_pool(name="const", bufs=1) as cp,
            tc.tile_pool(name="work", bufs=2) as wp,
        ):
            # ---- constants ----
            ident = cp.tile([P, P], f32, tag="ident")
            make_identity(nc, ident[:])
            iota = cp.tile([P, 1], i32, tag="iotai")
            nc.gpsimd.iota(iota[:], pattern=[[0, 1]], base=0, channel_multiplier=1)
            iotaf = cp.tile([P, 1], f32, tag="iotaf")
            nc.vector.tensor_copy(iotaf[:], iota[:])
            ones1 = cp.tile([1, P], f32, tag="ones1")
            nc.gpsimd.memset(ones1[:], 1.0)
            zt = cp.tile([P, C1], f32, tag="zt")
            nc.gpsimd.memset(zt[:], 0.0)

            w1aug = cp.tile([cfg.IN, C1 + A], f32, tag="w1aug")
            nc.sync.dma_start(w1aug[:], W1aug_d[:])
            w1ad = cp.tile([cfg.IN, A], f32, tag="w1ad")
            nc.sync.dma_start(w1ad[:], W1ad_d[:])
            w2h = [cp.tile([P, C2], f32, tag=f"w2h{b}", name=f"w2h{b}")
                   for b in range(2)]
            w2as = [cp.tile([P, A], f32, tag=f"w2as{b}", name=f"w2as{b}")
                    for b in range(2)]
            w2ad = [cp.tile([P, A], f32, tag=f"w2ad{b}", name=f"w2ad{b}")
                    for b in range(2)]
            for b in range(2):
                nc.sync.dma_start(w2h[b][:], W2h_d[b * P:(b + 1) * P, :])
                nc.sync.dma_start(w2as[b][:], W2as_d[b * P:(b + 1) * P, :])
                nc.sync.dma_start(w2ad[b][:], W2ad_d[b * P:(b + 1) * P, :])
            wm1 = cp.tile([64, 64], f32, tag="wm1")
            nc.sync.dma_start(wm1[:], Wm1_d[:])
            bm1 = cp.tile([64, 1], f32, tag="bm1")
            nc.sync.dma_start(bm1[:], bm1_d[:])
            wm2 = cp.tile([64, 2], f32, tag="wm2")
            nc.sync.dma_start(wm2[:], Wm2_d[:])

            # ---- zero the tail of t2shard (pad rows; feeds the sentinel row) ---
            ztail = min(2 * P, NSH)
            nc.sync.dma_start(t2shard[NSH - ztail:NSH - ztail + P, :], zt[:])
            if ztail > P:
                nc.sync.dma_start(t2shard[NSH - ztail + P:NSH, :], zt[:])

            # ---- phase 0: alpha_dst table for layer 1 (node-major) ----
            with tc.tile_pool(name="ps0", bufs=2, space="PSUM") as ps0:
                for i in range(NB):
                    xs = wp.tile([P, cfg.IN], f32, tag="p0_xs")
                    nc.sync.dma_start(xs[:], xshard[i * P:(i + 1) * P, :])
                    tp = ps0.tile([P, P], f32, tag="tp")
                    nc.tensor.transpose(tp[:], xs[:], ident[:])
                    xT = wp.tile([P, P], f32, tag="p0_xT")
                    nc.vector.tensor_copy(xT[:], tp[:])
                    ap = ps0.tile([P, A], f32, tag="aps")
                    nc.tensor.matmul(ap[:], lhsT=xT[:], rhs=w1ad[:],
                                     start=True, stop=True)
                    ad = wp.tile([P, A], f32, tag="p0_ad")
                    nc.vector.tensor_copy(ad[:], ap[:])
                    nc.sync.dma_start(ad1t[i * P:(i + 1) * P, :], ad[:])

            # ---- edge-tile phase (shared between the two GAT layers) ----
            def edge_phase(layer, t, pools):
                mi = wp.tile([P, 9], i32, tag="mi")
                nc.sync.dma_start(mi[:], meta_i[t])
                mc = wp.tile([P, G], f32, tag="mc")
                nc.sync.dma_start(mc[:], meta_c[t])
                mr = wp.tile([1, G * P], f32, tag="mr")
                nc.sync.dma_start(mr[:], meta_r[t:t + 1, :])

                adt = ad1t if layer == 1 else ad2t
                adx = wp.tile([P, A], f32, tag="adx")
                nc.gpsimd.indirect_dma_start(
                    out=adx[:], out_offset=None, in_=adt[:],
                    in_offset=bass.IndirectOffsetOnAxis(ap=mi[:, 8:9], axis=0))

                KW = cfg.IN if layer == 1 else C1
                gt = wp.tile([P, G * KW], f32, tag=f"g{layer}")
                src_tbl = xpad if layer == 1 else t2full
                # one gather per group: the HW DGE honors a single index
                # column per partition (multi-column index APs misbehave)
                for g in range(G):
                    nc.gpsimd.indirect_dma_start(
                        out=gt[:, g * KW:(g + 1) * KW],
                        out_offset=None, in_=src_tbl[:],
                        in_offset=bass.IndirectOffsetOnAxis(
                            ap=mi[:, g:g + 1], axis=0))

                sb_ps = pools["seg"].tile([P, G * P], f32, tag="segps")
                nc.tensor.matmul(sb_ps[:], lhsT=ones1[:], rhs=mr[:],
                                 start=True, stop=True)
                segb = wp.tile([P, G * P], f32, tag="segb")
                nc.vector.tensor_copy(segb[:], sb_ps[:])

                CO = C1 if layer == 1 else C2
                CH = cfg.CH1 if layer == 1 else cfg.CH2
                for g in range(G):
                    nkb = KW // P
                    gTs = []
                    for b in range(nkb):
                        tp = pools["tp"].tile([P, P], f32, tag="tp")
                        nc.tensor.transpose(
                            tp[:], gt[:, g * KW + b * P: g * KW + (b + 1) * P],
                            ident[:])
                        gT = wp.tile([P, P], f32, tag=f"gT{b}", name=f"gT{b}")
                        nc.vector.tensor_copy(gT[:], tp[:])
                        gTs.append(gT)

                    Eg = wp.tile([P, P], f32, tag="Eg")
                    nc.vector.tensor_tensor(
                        out=Eg[:], in0=iotaf[:].to_broadcast((P, P)),
                        in1=segb[:, g * P:(g + 1) * P], op=EQ)
                    Sg = wp.tile([P, P], f32, tag="Sg")
                    nc.vector.tensor_tensor(
                        out=Sg[:], in0=mc[:, g:g + 1].to_broadcast((P, P)),
                        in1=segb[:, g * P:(g + 1) * P], op=EQ)

                    if layer == 1:
                        hps = pools["h"].tile([P, C1 + A], f32, tag="hps")
                        nc.tensor.matmul(hps[:], lhsT=gTs[0][:], rhs=w1aug[:],
                                         start=True, stop=False,
                                         skip_group_check=True)
                        nc.tensor.matmul(hps[:, C1:C1 + A], lhsT=Eg[:],
                                         rhs=adx[:], start=False, stop=True,
                                         skip_group_check=True)
                        att_ap = hps[:, C1:C1 + A]
                        h_ap = hps[:, 0:C1]
                    else:
                        hps = pools["h"].tile([P, C2], f32, tag="hps")
                        att = pools["att"].tile([P, A], f32, tag="attps")
                        for b in range(2):
                            nc.tensor.matmul(hps[:], lhsT=gTs[b][:],
                                             rhs=w2h[b][:],
                                             start=(b == 0), stop=(b == 1))
                            nc.tensor.matmul(att[:], lhsT=gTs[b][:],
                                             rhs=w2as[b][:],
                                             start=(b == 0), stop=False,
                                             skip_group_check=True)
                        nc.tensor.matmul(att[:], lhsT=Eg[:], rhs=adx[:],
                                         start=False, stop=True,
                                         skip_group_check=True)
                        att_ap = att[:]
                        h_ap = hps[:]

                    # leaky-relu as 0.6x + 0.4|x| (one PSUM input per op), then exp
                    ab = wp.tile([P, A], f32, tag="ab")
                    nc.scalar.activation(ab[:], att_ap,
                                         mybir.ActivationFunctionType.Abs,
                                         scale=0.4)
                    ex = wp.tile([P, A], f32, tag="ex")
                    nc.vector.scalar_tensor_tensor(
                        out=ex[:], in0=att_ap, scalar=0.6, in1=ab[:],
                        op0=MUL, op1=mybir.AluOpType.add)
                    nc.scalar.activation(ex[:], ex[:], EXP)

                    M = wp.tile([P, A + CO], f32, tag=f"M{layer}",
                                name=f"M{layer}")
                    nc.vector.tensor_copy(M[:, 0:A], ex[:])
                    nc.vector.tensor_tensor(
                        out=M[:, A:A + CO].rearrange("p (h c) -> p h c", h=A),
                        in0=h_ap.rearrange("p (h c) -> p h c", h=A),
                        in1=ex[:, :, None].to_broadcast((P, A, CH)), op=MUL)

                    Pa = pools["pa"].tile([P, A], f32, tag="pa")
                    nc.tensor.matmul(Pa[:], lhsT=Sg[:], rhs=M[:, 0:A],
                                     start=True, stop=True)
                    Pb = pools["pb"].tile([P, CO], f32, tag="pb")
                    nc.tensor.matmul(Pb[:], lhsT=Sg[:], rhs=M[:, A:A + CO],
                                     start=True, stop=True)

                    r = wp.tile([P, A], f32, tag="r")
                    nc.vector.reciprocal(r[:], Pa[:])

                    if layer == 1:
                        h1r = wp.tile([P, C1], f32, tag="h1r")
                        nc.vector.tensor_tensor(
                            out=h1r[:].rearrange("p (h c) -> p h c", h=A),
                            in0=Pb[:].rearrange("p (h c) -> p h c", h=A),
                            in1=r[:, :, None].to_broadcast((P, A, CH)), op=MUL)
                        nc.scalar.activation(h1r[:], h1r[:], RELU)
                        nc.gpsimd.indirect_dma_start(
                            out=t2shard[:],
                            out_offset=bass.IndirectOffsetOnAxis(
                                ap=mi[:, 4 + g:5 + g], axis=0),
                            in_=h1r[:], in_offset=None,
                            bounds_check=NSH - 1, oob_is_err=False)
                    else:
                        tmp = wp.tile([P, C2], f32, tag="tmp2")
                        nc.vector.tensor_tensor(
                            out=tmp[:].rearrange("p (h c) -> p h c", h=A),
                            in0=Pb[:].rearrange("p (h c) -> p h c", h=A),
                            in1=r[:, :, None].to_broadcast((P, A, CH)), op=MUL)
                        o2 = wp.tile([P, 64], f32, tag="o2")
                        cview = bass.AP(tmp.tensor, tmp[:].offset,
                                        [list(tmp[:].ap[0]), [1, 64], [64, A]])
                        nc.vector.tensor_reduce(
                            out=o2[:], in_=cview, axis=mybir.AxisListType.X,
                            op=mybir.AluOpType.add)
                        nc.gpsimd.indirect_dma_start(
                            out=h2mt[:],
                            out_offset=bass.IndirectOffsetOnAxis(
                                ap=mi[:, 4 + g:5 + g], axis=0),
                            in_=o2[:], in_offset=None,
                            bounds_check=NSH - 1, oob_is_err=False)

            # ---- phase I: layer-1 edge tiles ----
            with (
                tc.tile_pool(name="psseg1", bufs=1, space="PSUM") as pseg,
                tc.tile_pool(name="pstp1", bufs=1, space="PSUM") as ptp,
                tc.tile_pool(name="psh1", bufs=2, space="PSUM") as ph,
                tc.tile_pool(name="pspa1", bufs=2, space="PSUM") as ppa,
                tc.tile_pool(name="pspb1", bufs=2, space="PSUM") as ppb,
            ):
                pools = dict(seg=pseg, tp=ptp, h=ph, pa=ppa, pb=ppb, att=None)
                for t in range(T):
                    edge_phase(1, t, pools)

            # ---- AllGather the layer-1 output node table ----
            nc.gpsimd.collective_compute(
                "AllGather", mybir.AluOpType.bypass,
                replica_groups=[list(range(cfg.n_cores))],
                ins=[t2shard[:].opt()], outs=[t2full[:].opt()])

            if DEBUG_OUTPUTS:
                for i in range(NB):
                    dbgt = wp.tile([P, C1], f32, tag="dbgt")
                    nc.sync.dma_start(dbgt[:], t2shard[i * P:(i + 1) * P, :])
                    nc.sync.dma_start(t2dbg[i * P:(i + 1) * P, :], dbgt[:])

            # ---- phase IIa: alpha_dst table for layer 2 ----
            with tc.tile_pool(name="ps2a", bufs=2, space="PSUM") as ps2a:
                for i in range(NB):
                    ts = wp.tile([P, C1], f32, tag="p2a_ts")
                    nc.sync.dma_start(ts[:], t2shard[i * P:(i + 1) * P, :])
                    ap2 = ps2a.tile([P, A], f32, tag="aps")
                    for b in range(2):
                        tp = ps2a.tile([P, P], f32, tag="tp")
                        nc.tensor.transpose(tp[:], ts[:, b * P:(b + 1) * P],
                                            ident[:])
                        tT = wp.tile([P, P], f32, tag="p2a_tT")
                        nc.vector.tensor_copy(tT[:], tp[:])
                        nc.tensor.matmul(ap2[:], lhsT=tT[:], rhs=w2ad[b][:],
                                         start=(b == 0), stop=(b == 1))
                    ad2 = wp.tile([P, A], f32, tag="p2a_ad")
                    nc.vector.tensor_copy(ad2[:], ap2[:])
                    nc.sync.dma_start(ad2t[i * P:(i + 1) * P, :], ad2[:])

            # ---- phase IIb: layer-2 edge tiles ----
            with (
                tc.tile_pool(name="psseg2", bufs=1, space="PSUM") as pseg,
                tc.tile_pool(name="pstp2", bufs=1, space="PSUM") as ptp,
                tc.tile_pool(name="psh2", bufs=2, space="PSUM") as ph,
                tc.tile_pool(name="psatt2", bufs=1, space="PSUM") as patt,
                tc.tile_pool(name="pspa2", bufs=1, space="PSUM") as ppa,
                tc.tile_pool(name="pspb2", bufs=2, space="PSUM") as ppb,
            ):
                pools = dict(seg=pseg, tp=ptp, h=ph, pa=ppa, pb=ppb, att=patt)
                for t in range(T):
                    edge_phase(2, t, pools)

            # ---- phase III: MLP node-major over the shard ----
            with tc.tile_pool(name="ps3", bufs=2, space="PSUM") as ps3:
                for i in range(NB):
                    hm = wp.tile([P, 64], f32, tag="p3_hm")
                    nc.sync.dma_start(hm[:], h2mt[i * P:(i + 1) * P, :])
                    tp = ps3.tile([64, P], f32, tag="tp64")
                    nc.tensor.transpose(tp[:], hm[:], ident[:])
                    hmT = wp.tile([64, P], f32, tag="p3_hmT")
                    nc.vector.tensor_copy(hmT[:], tp[:])
                    m1 = ps3.tile([64, P], f32, tag="m1ps")
                    nc.tensor.matmul(m1[:], lhsT=wm1[:], rhs=hmT[:],
                                     start=True, stop=True)
                    hr = wp.tile([64, P], f32, tag="p3_hr")
                    nc.scalar.activation(hr[:], m1[:], RELU, bias=bm1[:, 0:1])
                    m2 = ps3.tile([2, P], f32, tag="m2ps")
                    nc.tensor.matmul(m2[:], lhsT=wm2[:], rhs=hr[:],
                                     start=True, stop=True)
                    ob = wp.tile([2, P], f32, tag="p3_ob")
                    nc.vector.tensor_copy(ob[:], m2[:])
                    nc.sync.dma_start(
                        outf[i * P:(i + 1) * P, :].rearrange("n c -> c n"),
                        ob[:])

    nc.compile()
    return nc


def make_in_maps(x, edge_index, weights, cfg):
    xpad, metas = host_prep(x, edge_index, cfg)
    in_maps = []
    for c in range(cfg.n_cores):
        mi, mc, mr = metas[c]
        m = dict(
            xpad=xpad,
            xshard=np.ascontiguousarray(
                xpad[c * cfg.nshard:(c + 1) * cfg.nshard]),
            meta_i=mi, meta_c=mc, meta_r=mr,
            **{k: np.ascontiguousarray(v) for k, v in weights.items()},
        )
        in_maps.append(m)
    return in_maps


# ----------------------------------------------------------------- execution


class Runner:
    """Persistent jitted SPMD executor (mirrors bass2jax.run_bass_via_pjrt)."""

    def __init__(self, nc, n_cores):
        import jax
        from concourse import bass2jax, mybir
        from jax.sharding import Mesh, PartitionSpec, NamedSharding
        from jax.experimental.shard_map import shard_map

        bass2jax.install_neuronx_cc_hook()
        self.n_cores = n_cores
        self.jax = jax

        part_name = (nc.partition_id_tensor.name if nc.partition_id_tensor
                     else None)
        in_names, out_names, out_avals, zero_outs = [], [], [], []
        for alloc in nc.m.functions[0].allocations:
            if not isinstance(alloc, mybir.MemoryLocationSet):
                continue
            name = alloc.memorylocations[0].name
            if alloc.kind == "ExternalInput":
                if name != part_name:
                    in_names.append(name)
            elif alloc.kind == "ExternalOutput":
                shape = tuple(alloc.tensor_shape)
                dtype = mybir.dt.np(alloc.dtype)
                out_names.append(name)
                out_avals.append(jax.core.ShapedArray(shape, dtype))
                zero_outs.append(np.zeros(shape, dtype))
        self.in_names = list(in_names)
        self.out_names = out_names
        self.out_avals = out_avals
        self.zero_outs = zero_outs
        n_params = len(in_names)
        n_outs = len(out_names)
        all_names = in_names + out_names
        if part_name is not None:
            all_names.append(part_name)

        from concourse.bass2jax import _bass_exec_p, partition_id_tensor

        def _body(*args):
            operands = list(args)
            if part_name is not None:
                operands.append(partition_id_tensor())
            outs = _bass_exec_p.bind(
                *operands,
                out_avals=tuple(out_avals),
                in_names=tuple(all_names),
                out_names=tuple(out_names),
                lowering_input_output_aliases=(),
                sim_require_finite=False,
                sim_require_nnan=False,
                nc=nc,
            )
            return tuple(outs)

        devices = jax.devices()[:n_cores]
        self.mesh = Mesh(np.asarray(devices), ("core",))
        self.spec = NamedSharding(self.mesh, PartitionSpec("core"))
        in_specs = (PartitionSpec("core"),) * (n_params + n_outs)
        out_specs = (PartitionSpec("core"),) * n_outs
        donate = tuple(range(n_params, n_params + n_outs))
        self.fn = jax.jit(
            shard_map(_body, mesh=self.mesh, in_specs=in_specs,
                      out_specs=out_specs, check_rep=False),
            donate_argnums=donate, keep_unused=True)

    def put_inputs(self, in_maps):
        """device_put the concatenated per-core inputs once (outside timing)."""
        jax = self.jax
        self.dev_in = [
            jax.device_put(
                np.concatenate([np.asarray(in_maps[c][n])
                                for c in range(self.n_cores)], axis=0),
                self.spec)
            for n in self.in_names
        ]
        jax.block_until_ready(self.dev_in)

    def run(self):
        jax = self.jax
        zo = [jax.device_put(
            np.zeros((self.n_cores * z.shape[0], *z.shape[1:]), z.dtype),
            self.spec) for z in self.zero_outs]
        jax.block_until_ready(zo)
        t0 = time.perf_counter_ns()
        outs = self.fn(*self.dev_in, *zo)
        jax.block_until_ready(outs)
        t1 = time.perf_counter_ns()
        res = {
            name: np.asarray(outs[i]).reshape(
                self.n_cores, *self.out_avals[i].shape)
            for i, name in enumerate(self.out_names)
        }
        return res, t1 - t0


_CACHE = {}
LAST_EXEC_NS = None


def kernel(x, edge_index, W1, a1_src, a1_dst, b1, W2, a2_src, a2_dst, b2,
           Wm1, bm1, Wm2, bm2):
    global LAST_EXEC_NS
    assert float(np.abs(np.asarray(b1)).max()) == 0.0, \
        "nonzero b1 unsupported by this kernel build"

    cfg = Cfg(n_real=N_REAL, n_cores=N_CORES, nshard=NSHARD, T=0)
    weights = prep_weights(W1, a1_src, a1_dst, W2, a2_src, a2_dst,
                           Wm1, bm1, b2, Wm2, cfg)
    in_maps = make_in_maps(x, edge_index, weights, cfg)

    key = ("prog", cfg.T)
    if key not in _CACHE:
        nc = build_program(cfg)
        _CACHE.clear()
        _CACHE[key] = Runner(nc, cfg.n_cores)
    runner = _CACHE[key]
    runner.put_inputs(in_maps)

    res, _ = runner.run()          # warm-up (includes compile on first call)
    res, dt = runner.run()         # timed warm run
    LAST_EXEC_NS = dt

    out = res["outf"].reshape(cfg.npad, 2)[:cfg.n_real]
    return (out + np.asarray(bm2, np.float32)).astype(np.float32)
